# revision 13
# baseline (speedup 1.0000x reference)
"""Trainium2 Bass kernel for COMPoolNet (hierarchical GCN + COMSAGPool).

Sharding: data-parallel over the 256-graph batch across 8 NeuronCores
(32 graphs/core); small weights replicated.

Device strategy: message passing is reformulated as dense per-graph
adjacency matmuls on the TensorEngine:

    gcn_conv(x) = r_in * (A @ (r_out * (x @ W))) + b

The graph-contiguous edge list (256 nodes/graph) is converted on the host
into per-graph dense adjacency count matrices AT[s,d] (index-only
preprocessing, analogous to building DMA descriptors); all floating-point
math runs on device.  Top-k pooling is computed on device via an all-pairs
rank compare with exact tie repair; pool compaction, pooled adjacency
(A2 = S A S^T) and the avg/max readouts are realized as selection-matrix
matmuls on the PE.  The global softmax over node scores is reduced
per-core on device (max + sum of exp) and merged across shards on the
host as part of unsharding.
"""

import sys
import numpy as np

sys.path.insert(0, '/opt/trn_rl_repo')

from contextlib import ExitStack  # noqa: E402

import concourse.bass as bass  # noqa: E402,F401
import concourse.tile as tile  # noqa: E402
from concourse import bacc, mybir  # noqa: E402
from concourse import bass_utils  # noqa: E402

F32 = mybir.dt.float32
AF = mybir.ActivationFunctionType
ALU = mybir.AluOpType
AX = mybir.AxisListType

B, NP, D, C, EPG = 256, 256, 128, 10, 4096
K1, K2, K3 = 128, 64, 32
NCORES = 8
G = B // NCORES          # 32 graphs per core
BIG = 65536.0  # must keep rank integers exact when added in fp32

STAGES = [
    dict(n=256, k=128, P=128),
    dict(n=128, k=64, P=128),
    dict(n=64, k=32, P=64),
]

WSIZES = [("W1", 128, 128), ("W2", 128, 128), ("W3", 128, 128),
          ("Wg0a", 128, 128), ("Wg0b", 128, 128), ("Wg1", 128, 64),
          ("Wg2", 64, 10), ("Wp0", 128, 64), ("Wp1", 64, 32),
          ("Wp2", 32, 10), ("Ws", 128, 1)]
BSIZES = [("b1", 128), ("b2", 128), ("b3", 128), ("bg0", 128), ("bg1", 64),
          ("bg2", 10), ("bp0", 64), ("bp1", 32), ("bp2", 10), ("bs", 1)]
WOFF = {}
_o = 0
for _nm, _r, _c in WSIZES:
    WOFF[_nm] = _o
    _o += _c
WCOLS = _o
BOFF = {}
_o = 0
for _nm, _c in BSIZES:
    BOFF[_nm] = _o
    _o += _c
BCOLS = _o


DEBUG_SHAPES = {
    "s1": [32, 256], "rank1": [32, 256], "kept1": [32, 256],
    "dest1": [32, 256], "gate1": [32, 256],
    "o1": [128, 256], "x2": [128, 128], "at2": [128, 128],
    "y1": [128, 64], "s2": [32, 128], "kept2": [32, 128],
    "o2": [128, 128], "x3": [64, 128], "at3": [64, 64],
    "s3": [32, 64], "o3": [64, 128],
    "hgdavg": [128, 32], "hgdmax": [128, 32],
    "hgcavg": [128, 32], "hgcmax": [128, 32],
}
DEBUG = []          # names enabled for the next _build_program()


def build_body(ctx, tc, outs, ins, dbg_aps=None):
    dbg_aps = dbg_aps or {}

    def dbg(name, ap):
        if name in dbg_aps:
            tc.nc.sync.dma_start(dbg_aps[name][:], ap)

    nc = tc.nc
    (bigx_d, bigat_d, dego_col_d, degi_col_d, degi_byg_d,
     wpack_d, bpack_d, gsel_d, id128_d, iota_d) = ins
    (scoresT_d, scorescT_d, hg3comT_d, npredT_d, e1_d, msum_d) = outs

    # ---------------- pools ----------------
    const = ctx.enter_context(tc.tile_pool(name="const", bufs=1))
    bigp = ctx.enter_context(tc.tile_pool(name="bigp", bufs=1))
    big1 = ctx.enter_context(tc.tile_pool(name="big1", bufs=1))
    work = ctx.enter_context(tc.tile_pool(name="work", bufs=1))
    wave = ctx.enter_context(tc.tile_pool(name="wave", bufs=2))
    smal = ctx.enter_context(tc.tile_pool(name="smal", bufs=3))
    psT = ctx.enter_context(tc.tile_pool(name="psT", bufs=2, space="PSUM"))
    psS = ctx.enter_context(tc.tile_pool(name="psS", bufs=2, space="PSUM"))
    psZ = ctx.enter_context(tc.tile_pool(name="psZ", bufs=2, space="PSUM"))
    psC = ctx.enter_context(tc.tile_pool(name="psC", bufs=2, space="PSUM"))

    # ---------------- constants ----------------
    wpack = const.tile([128, WCOLS], F32, tag="wpack")
    nc.sync.dma_start(wpack[:], wpack_d[:])
    bpack = const.tile([1, BCOLS], F32, tag="bpack")
    nc.sync.dma_start(bpack[:], bpack_d[:])
    gsel = const.tile([32, 128], F32, tag="gsel")
    nc.sync.dma_start(gsel[:], gsel_d[:])
    id128 = const.tile([128, 128], F32, tag="id128")
    nc.sync.dma_start(id128[:], id128_d[:])
    iota = const.tile([128, 128], F32, tag="iota")
    nc.sync.dma_start(iota[:], iota_d[:])
    ones1 = const.tile([1, 512], F32, tag="ones1")
    nc.vector.memset(ones1[:], 1.0)

    WROWS = dict((nm, r) for nm, r, c in WSIZES)
    WCOL = dict((nm, c) for nm, r, c in WSIZES)

    def W(nm):
        return wpack[0:WROWS[nm], WOFF[nm]:WOFF[nm] + WCOL[nm]]

    def Brow(nm):
        nn = dict(BSIZES)[nm]
        return bpack[:, BOFF[nm]:BOFF[nm] + nn]

    bias_cols = {}
    for nm in ("bg0", "bg1", "bg2", "bp0", "bp1", "bp2"):
        nn = dict(BSIZES)[nm]
        pt = psT.tile([nn, 1], F32, tag="T")
        nc.tensor.transpose(pt[:], Brow(nm), id128[0:1, 0:1])
        t = const.tile([nn, 1], F32, tag=f"bc_{nm}")
        nc.scalar.copy(t[:], pt[:])
        bias_cols[nm] = t

    def rsqrt_of(dcol, shape, otag):
        """rsqrt(max(d,1)) as sqrt(1/d) + 2 Newton steps (d integer counts)."""
        dc = work.tile(shape, F32, tag="rsq_dc")
        nc.vector.tensor_scalar(dc[:], dcol, 1.0, None, ALU.max)
        rec = work.tile(shape, F32, tag="rsq_rec")
        nc.vector.reciprocal(rec[:], dc[:])
        r = work.tile(shape, F32, tag="rsq_r0")
        nc.scalar.activation(r[:], rec[:], AF.Sqrt)
        for it in range(2):
            r2 = work.tile(shape, F32, tag=f"rsq_r2{it}")
            nc.vector.tensor_tensor(r2[:], r[:], r[:], ALU.mult)
            nc.vector.tensor_tensor(r2[:], r2[:], dc[:], ALU.mult)
            nc.vector.scalar_tensor_tensor(r2[:], r2[:], -0.5, r[:],
                                           ALU.mult, ALU.mult)
            rn = work.tile(shape, F32,
                           tag=(otag if it == 1 else f"rsq_rn{it}"))
            nc.vector.scalar_tensor_tensor(rn[:], r[:], 1.5, r2[:],
                                           ALU.mult, ALU.add)
            r = rn
        return r

    # ---------------- big inputs ----------------
    # tag "bigx0" slot is reused for the gated dis-selection matrix (stg)
    # after X0 dies at the stage-1 residual; tag "bigat1" slot is reused for
    # the stage-2 pooled adjacency.
    bigx0 = bigp.tile([128, 64 * 128], F32, tag="bigx0")
    nc.sync.dma_start(bigx0[:], bigx_d[:])

    dego_c = work.tile([128, 64], F32, tag="deg_oc")
    nc.sync.dma_start(dego_c[:], dego_col_d[:])
    degi_c = work.tile([128, 64], F32, tag="deg_ic")
    nc.sync.dma_start(degi_c[:], degi_col_d[:])
    degi_b = work.tile([32, 256], F32, tag="deg_ib")
    nc.sync.dma_start(degi_b[:], degi_byg_d[:])
    r_out_1 = rsqrt_of(dego_c[:], [128, 64], "r_o1")
    r_in_1 = rsqrt_of(degi_c[:], [128, 64], "r_i1")
    r_in_1b = rsqrt_of(degi_b[:], [32, 256], "r_i1b")

    hgT = {}
    for side in ("dis", "com"):
        for kind in ("avg", "max"):
            t = work.tile([128, 32], F32, tag=f"hgT_{side}_{kind}")
            nc.vector.memset(t[:], 0.0)
            hgT[(side, kind)] = t

    st = {"X": bigx0, "AT": None, "ro_c": r_out_1, "ri_c": r_in_1,
          "ri_b": r_in_1b}
    WNM = ["W1", "W2", "W3"]
    BNM = ["b1", "b2", "b3"]

    e1_tile = msum_tile = None
    hg3c_avg = hg3c_max = None
    bigO_last = None

    for s_i, SP in enumerate(STAGES):
        n, k, P = SP["n"], SP["k"], SP["P"]
        cpg = n // P if n >= P else 1        # 2,1,1
        nb = G * cpg                         # node blocks
        last = s_i == 2

        bigX, bigAT = st["X"], st["AT"]
        ro_c, ri_c, ri_b = st["ro_c"], st["ri_c"], st["ri_b"]

        gcols = cpg * cpg * P               # at-columns per graph

        def at_load(g0, gn, tag="atw"):
            """stage-1 only: DMA gn graphs' AT blocks into a wave tile."""
            t = wave.tile([128, 4 * 512], F32, tag=tag)
            nc.sync.dma_start(t[:, 0:gn * gcols],
                              bigat_d[:, g0 * gcols:(g0 + gn) * gcols])
            return t

        def atblk_of(t, g0, g, cs, cd):
            """block (g,cs,cd) from wave tile t holding graphs from g0
            (stage 2/3: t is the resident AT, g0=0)."""
            o = (g - g0) * gcols + (cs * cpg + cd) * P
            return t[0:P, o:o + P]

        # ---------- conv layer: fused wave loop ----------
        # out = X + relu(r_in * (A @ (r_out * X @ W)) + b)
        pbr = psS.tile([128, 512], F32, tag="S")
        nc.tensor.matmul(pbr[0:P, 0:128], ones1[:, 0:P], Brow(BNM[s_i]),
                         start=True, stop=True)
        brep = work.tile([128, 128], F32, tag="brep")
        nc.scalar.copy(brep[0:P, :], pbr[0:P, 0:128])
        bigO = big1.tile([128, 64 * 128], F32, tag="bigO")
        bigOv = bigO[0:P, 0:nb * 128]
        WPG = max(1, 8 // cpg)               # graphs per wave (8 blocks)
        for g0 in range(0, G, WPG):
            blo = g0 * cpg
            bn = WPG * cpg
            atw = at_load(g0, WPG) if s_i == 0 else bigAT
            atg0 = g0 if s_i == 0 else 0
            xw = wave.tile([128, 8 * 128], F32, tag="xw")
            nc.vector.tensor_tensor(
                xw[0:P, 0:bn * 128].rearrange("p (b f) -> p b f", b=bn),
                bigX[0:P, blo * 128:(blo + bn) * 128].rearrange(
                    "p (b f) -> p b f", b=bn),
                ro_c[:, blo:blo + bn].unsqueeze(2).broadcast_to([P, bn, 128]),
                ALU.mult)
            zw = wave.tile([128, 8 * 128], F32, tag="zw")
            for b in range(bn):
                ptr = psT.tile([128, 128], F32, tag="T")
                nc.tensor.transpose(ptr[:, 0:P],
                                    xw[0:P, b * 128:(b + 1) * 128],
                                    id128[0:P, 0:P])
                xsT = smal.tile([128, 128], F32, tag="xsT")
                nc.scalar.copy(xsT[:, 0:P], ptr[:, 0:P])
                if b % 4 == 0:
                    pz = psZ.tile([128, 512], F32, tag="Z")
                nc.tensor.matmul(pz[0:P, (b % 4) * 128:(b % 4) * 128 + 128],
                                 xsT[:, 0:P], W(WNM[s_i]), start=True,
                                 stop=True)
                if b % 4 == 3:
                    nc.scalar.copy(zw[0:P, (b - 3) * 128:(b + 1) * 128],
                                   pz[0:P, :])
            pidx = 0
            pc = None
            for gg in range(WPG):
                g = g0 + gg
                for cd in range(cpg):
                    if pidx % 4 == 0:
                        pc = psC.tile([128, 512], F32, tag="Cp")
                    sl = pc[0:P, (pidx % 4) * 128:(pidx % 4) * 128 + 128]
                    for cs in range(cpg):
                        nc.tensor.matmul(sl, atblk_of(atw, atg0, g, cs, cd),
                                         zw[0:P, (gg * cpg + cs) * 128:
                                            (gg * cpg + cs + 1) * 128],
                                         start=(cs == 0),
                                         stop=(cs == cpg - 1))
                    pidx += 1
                    if pidx % 4 == 0 or pidx == bn:
                        lo = ((pidx - 1) // 4) * 4
                        cnt = pidx - lo
                        nc.vector.tensor_tensor(
                            bigOv[:, (blo + lo) * 128:(blo + lo + cnt) * 128]
                            .rearrange("p (b f) -> p b f", b=cnt),
                            pc[0:P, 0:cnt * 128].rearrange(
                                "p (b f) -> p b f", b=cnt),
                            ri_c[:, blo + lo:blo + lo + cnt].unsqueeze(2)
                            .broadcast_to([P, cnt, 128]),
                            ALU.mult)
        nc.vector.tensor_tensor(
            bigOv[:].rearrange("p (b f) -> p b f", b=nb),
            bigOv[:].rearrange("p (b f) -> p b f", b=nb),
            brep[0:P, :].unsqueeze(1).broadcast_to([P, nb, 128]),
            ALU.add)
        nc.vector.scalar_tensor_tensor(bigOv[:], bigOv[:], 0.0,
                                       bigX[0:P, 0:nb * 128],
                                       ALU.max, ALU.add)

        if s_i == 0:
            dbg("o1", bigOv[:, 0:256])
        elif s_i == 1:
            dbg("o2", bigOv[:, 0:128])
        else:
            dbg("o3", bigOv[:, 0:128])

        def oblk(g, cs, _b=bigO, _cpg=cpg, _P=P):
            return _b[0:_P, (g * _cpg + cs) * 128:(g * _cpg + cs + 1) * 128]

        # ---------- y = r_out * (out @ Ws) ----------
        py = psZ.tile([128, 512], F32, tag="Z")
        for b in range(nb):
            ptr = psT.tile([128, 128], F32, tag="T")
            nc.tensor.transpose(ptr[:, 0:P],
                                bigOv[:, b * 128:(b + 1) * 128],
                                id128[0:P, 0:P])
            oT = smal.tile([128, 128], F32, tag="xsT")
            nc.scalar.copy(oT[:, 0:P], ptr[:, 0:P])
            nc.tensor.matmul(py[0:P, b:b + 1], oT[:, 0:P], W("Ws"),
                             start=True, stop=True)
        ycols = work.tile([128, 64], F32, tag="ycols")
        nc.vector.tensor_tensor(ycols[0:P, 0:nb], py[0:P, 0:nb],
                                ro_c[:, 0:nb], ALU.mult)
        if s_i == 0:
            dbg("y1", ycols[:])

        # ---------- score rows ----------
        s_byg = work.tile([32, 256], F32, tag="sbyg")
        for g0s in range(0, G, 4):
            atw = at_load(g0s, 4) if s_i == 0 else bigAT
            atg0 = g0s if s_i == 0 else 0
            srow4 = wave.tile([1, 1024], F32, tag="srow4")
            for g in range(g0s, g0s + 4):
                ps = psS.tile([128, 512], F32, tag="S")
                for cs in range(cpg):
                    o = (g - atg0) * gcols + (cs * cpg) * P
                    nc.tensor.matmul(
                        ps[0:1, 0:n],
                        ycols[0:P, g * cpg + cs:g * cpg + cs + 1],
                        atw[0:P, o:o + cpg * P],
                        start=(cs == 0), stop=(cs == cpg - 1))
                nc.scalar.copy(srow4[:, (g - g0s) * n:(g - g0s + 1) * n],
                               ps[0:1, 0:n])
            nc.sync.dma_start(
                s_byg[g0s:g0s + 4, 0:n],
                srow4[:, 0:4 * n].rearrange("a (g f) -> a g f", g=4))
        sv = s_byg[:, 0:n]
        nc.vector.tensor_tensor(sv, sv, ri_b[0:32, 0:n], ALU.mult)
        pbs = psS.tile([128, 512], F32, tag="S")
        nc.tensor.matmul(pbs[0:32, 0:1], ones1[:, 0:32], Brow("bs"),
                         start=True, stop=True)
        bs_sb = work.tile([32, 1], F32, tag="bs_sb")
        nc.scalar.copy(bs_sb[:], pbs[0:32, 0:1])
        nc.vector.tensor_scalar(sv, sv, bs_sb[:], None, ALU.add)

        if s_i == 0:
            dbg("s1", sv)
        elif s_i == 1:
            dbg("s2", sv)
        else:
            dbg("s3", sv)

        # ---------- softmax pieces (stage 1) ----------
        if s_i == 0:
            mx = work.tile([32, 1], F32, tag="sm_mx")
            nc.vector.tensor_reduce(mx[:], sv, AX.X, ALU.max)
            pmx = psS.tile([128, 512], F32, tag="S")
            nc.tensor.transpose(pmx[0:1, 0:32], mx[:], id128[0:32, 0:32])
            mxr = work.tile([1, 32], F32, tag="sm_mxr")
            nc.scalar.copy(mxr[:], pmx[0:1, 0:32])
            mxg = work.tile([1, 1], F32, tag="sm_mxg")
            nc.vector.tensor_reduce(mxg[:], mxr[:], AX.X, ALU.max)
            pmg = psS.tile([128, 512], F32, tag="S")
            nc.tensor.matmul(pmg[0:32, 0:1], ones1[:, 0:32], mxg[:],
                             start=True, stop=True)
            nmc = work.tile([32, 1], F32, tag="sm_nmc")
            nc.scalar.copy(nmc[:], pmg[0:32, 0:1])
            nc.vector.tensor_scalar_mul(nmc[:], nmc[:], -1.0)
            e1_tile = work.tile([32, 256], F32, tag="e1")
            esum = work.tile([32, 1], F32, tag="sm_esum")
            nc.scalar.activation(e1_tile[:], sv, AF.Exp, bias=nmc[:],
                                 accum_out=esum[:])
            pes = psS.tile([128, 512], F32, tag="S")
            nc.tensor.transpose(pes[0:1, 0:32], esum[:], id128[0:32, 0:32])
            esr = work.tile([1, 32], F32, tag="sm_esr")
            nc.scalar.copy(esr[:], pes[0:1, 0:32])
            esg = work.tile([1, 1], F32, tag="sm_esg")
            nc.vector.tensor_reduce(esg[:], esr[:], AX.X, ALU.add)
            msum_tile = work.tile([1, 2], F32, tag="msum")
            nc.vector.tensor_copy(msum_tile[:, 0:1], mxg[:])
            nc.vector.tensor_copy(msum_tile[:, 1:2], esg[:])

        # ---------- top-k ----------
        psr = psS.tile([128, 512], F32, tag="S")
        nc.tensor.matmul(psr[:, 0:n], gsel[:], sv, start=True, stop=True)
        srep = work.tile([128, 256], F32, tag="srep")
        nc.scalar.copy(srep[:, 0:n], psr[:, 0:n])
        sresh = work.tile([128, 64], F32, tag="sresh")
        nc.sync.dma_start(sresh[:, 0:n // 4],
                          sv.rearrange("g (q f) -> g q f", q=4))
        rank_r = work.tile([128, 64], F32, tag="rankr")
        nI = n // 4
        npass = 32 if s_i == 0 else (8 if s_i == 1 else 2)
        ih = nI // npass
        for h in range(npass):
            gt = big1.tile([128, 2 * 256], F32, tag="gt")
            il = slice(h * ih, (h + 1) * ih)
            gtv = gt[:, 0:ih * n]
            # gt[p, i, j] = (s_j > s_i)
            nc.vector.tensor_tensor(
                gtv.rearrange("p (i j) -> p i j", j=n),
                srep[:, 0:n].unsqueeze(1).broadcast_to([128, ih, n]),
                sresh[:, il].unsqueeze(2).broadcast_to([128, ih, n]),
                ALU.is_gt)
            nc.vector.tensor_reduce(rank_r[:, il],
                                    gtv.rearrange("p (i j) -> p i j", j=n),
                                    AX.X, ALU.add)
        rank_b = work.tile([32, 256], F32, tag="rankb")
        nc.sync.dma_start(
            rank_b[:, 0:n].rearrange("g (q f) -> g q f", q=4),
            rank_r[:, 0:nI])

        rbv = rank_b[:, 0:n]
        kept0 = work.tile([32, 256], F32, tag="kept0")
        k0v = kept0[:, 0:n]
        nc.vector.tensor_scalar(k0v, rbv, float(k), None, ALU.is_lt)
        t1 = work.tile([32, 256], F32, tag="tk1")
        t1v = t1[:, 0:n]
        nc.vector.tensor_scalar(t1v, rbv, BIG, None, ALU.add)
        nc.vector.tensor_tensor(t1v, t1v, k0v, ALU.mult)
        nc.vector.tensor_scalar(t1v, t1v, -BIG, None, ALU.add)
        rb = work.tile([32, 1], F32, tag="rbm")
        nc.vector.tensor_reduce(rb[:], t1v, AX.X, ALU.max)
        grp = work.tile([32, 256], F32, tag="grp")
        gv = grp[:, 0:n]
        nc.vector.tensor_scalar(gv, rbv, rb[:], None, ALU.is_equal)
        zz = work.tile([32, 256], F32, tag="zzz")
        nc.vector.memset(zz[:], 0.0)
        cnt = work.tile([32, 256], F32, tag="cntt")
        nc.vector.tensor_tensor_scan(cnt[:, 0:n], gv, zz[:, 0:n], 0.0,
                                     ALU.add, ALU.add)
        lim = work.tile([32, 1], F32, tag="lim")
        nc.vector.tensor_scalar(lim[:], rb[:], -1.0, float(k), ALU.mult,
                                ALU.add)
        sel = work.tile([32, 256], F32, tag="sel")
        nc.vector.tensor_scalar(sel[:, 0:n], cnt[:, 0:n], lim[:], None,
                                ALU.is_le)
        kept = work.tile([32, 256], F32, tag="kept")
        kv = kept[:, 0:n]
        nc.vector.scalar_tensor_tensor(kv, sel[:, 0:n], -1.0, gv,
                                       ALU.add, ALU.mult)
        nc.vector.tensor_tensor(kv, kv, k0v, ALU.add)

        def mkdest(mask, tagp):
            dd = work.tile([32, 256], F32, tag=tagp)
            dv = dd[:, 0:n]
            nc.vector.tensor_tensor_scan(dv, mask, zz[:, 0:n], 0.0,
                                         ALU.add, ALU.add)
            nc.vector.tensor_scalar(dv, dv, -1.0, None, ALU.add)
            nc.vector.tensor_tensor(dv, dv, mask, ALU.mult)
            tm = work.tile([32, 256], F32, tag=f"{tagp}_m")
            nc.vector.tensor_scalar(tm[:, 0:n], mask, -999.0, 999.0,
                                    ALU.mult, ALU.add)
            nc.vector.tensor_tensor(dv, dv, tm[:, 0:n], ALU.add)
            return dd

        if s_i == 0:
            dbg("rank1", rbv)
            dbg("kept1", kv)
        elif s_i == 1:
            dbg("kept2", kv)
        dest = mkdest(kv, "dest")
        keptc = work.tile([32, 256], F32, tag="keptc")
        nc.vector.tensor_scalar(keptc[:, 0:n], kv, -1.0, 1.0, ALU.mult,
                                ALU.add)
        destc = mkdest(keptc[:, 0:n], "destc")
        gate = work.tile([32, 256], F32, tag="gate")
        nc.scalar.activation(gate[:, 0:n], sv, AF.Tanh)
        if s_i == 0:
            dbg("dest1", dest[:, 0:n])
            dbg("gate1", gate[:, 0:n])

        # ---------- bridge byg -> cols [P, nb] ----------
        cols = {}
        for nm, src_t in (("gate", gate), ("dest", dest), ("destc", destc)):
            ct = work.tile([128, 64], F32, tag=f"col_{nm}")
            for cs in range(cpg):
                pt2 = psT.tile([128, 128], F32, tag="T")
                nc.tensor.transpose(pt2[0:P, 0:32],
                                    src_t[:, cs * P:(cs + 1) * P],
                                    id128[0:32, 0:32])
                if cpg == 1:
                    nc.scalar.copy(ct[0:P, 0:nb], pt2[0:P, 0:32])
                else:
                    nc.scalar.copy(
                        ct[0:P, 0:nb].rearrange("p (g c) -> p g c",
                                                c=cpg)[:, :, cs],
                        pt2[0:P, 0:32])
            cols[nm] = ct

        # ---------- gated dis-selection matrix (in X0's slot) ----------
        stg = bigp.tile([128, 64 * 128], F32, tag="bigx0")
        stgv = stg[0:P, 0:nb * k]
        nc.vector.tensor_tensor(
            stgv.rearrange("p (b j) -> p b j", b=nb),
            iota[0:P, 0:k].unsqueeze(1).broadcast_to([P, nb, k]),
            cols["dest"][0:P, 0:nb].unsqueeze(2).broadcast_to([P, nb, k]),
            ALU.is_equal)
        nc.vector.tensor_tensor(
            stgv.rearrange("p (b j) -> p b j", b=nb),
            stgv.rearrange("p (b j) -> p b j", b=nb),
            cols["gate"][0:P, 0:nb].unsqueeze(2).broadcast_to([P, nb, k]),
            ALU.mult)

        def stgblk(g, cs, _s=stg, _cpg=cpg, _P=P, _k=k):
            return _s[0:_P, (g * _cpg + cs) * _k:(g * _cpg + cs + 1) * _k]

        # ---------- readouts ----------
        if last:
            hg3c_avg = work.tile([128, 32], F32, tag="hg3c_avg")
            hg3c_max = work.tile([128, 32], F32, tag="hg3c_max")

        def accum_readout(pp, gn, g0, kk, side):
            red = pp[:, 0:gn * kk].rearrange("p (g j) -> p g j", g=gn)
            sm = work.tile([128, 32], F32, tag=f"rosum_{side}")
            nc.vector.tensor_reduce(sm[:, 0:gn], red, AX.X, ALU.add)
            mxx = work.tile([128, 32], F32, tag=f"romax_{side}")
            nc.vector.tensor_reduce(mxx[:, 0:gn], red, AX.X, ALU.max)
            if last and side == "com":
                nc.vector.tensor_scalar(hg3c_avg[:, g0:g0 + gn], sm[:, 0:gn],
                                        1.0 / kk, None, ALU.mult)
                nc.vector.tensor_copy(hg3c_max[:, g0:g0 + gn], mxx[:, 0:gn])
            nc.vector.scalar_tensor_tensor(
                hgT[(side, "avg")][:, g0:g0 + gn], sm[:, 0:gn], 1.0 / kk,
                hgT[(side, "avg")][:, g0:g0 + gn], ALU.mult, ALU.add)
            nc.vector.tensor_tensor(
                hgT[(side, "max")][:, g0:g0 + gn],
                hgT[(side, "max")][:, g0:g0 + gn], mxx[:, 0:gn], ALU.add)

        per = 512 // k
        for g0 in range(0, G, per):
            gn = min(per, G - g0)
            pp = psC.tile([128, 512], F32, tag="Cp")
            for gg in range(gn):
                g = g0 + gg
                slp = pp[:, gg * k:(gg + 1) * k]
                for cs in range(cpg):
                    nc.tensor.matmul(slp, oblk(g, cs), stgblk(g, cs),
                                     start=(cs == 0), stop=(cs == cpg - 1))
            accum_readout(pp, gn, g0, k, "dis")

        # com side: gated com-selection built in per-group waves
        kkc = n - k
        perc = 512 // kkc
        for g0 in range(0, G, perc):
            gn = min(perc, G - g0)
            bnw = gn * cpg
            stw = wave.tile([128, 8 * 128], F32, tag="xw")
            cslice = slice(g0 * cpg, g0 * cpg + bnw)
            stwv = stw[0:P, 0:bnw * kkc]
            nc.vector.tensor_tensor(
                stwv.rearrange("p (b j) -> p b j", b=bnw),
                iota[0:P, 0:kkc].unsqueeze(1).broadcast_to([P, bnw, kkc]),
                cols["destc"][0:P, cslice].unsqueeze(2)
                .broadcast_to([P, bnw, kkc]),
                ALU.is_equal)
            nc.vector.tensor_tensor(
                stwv.rearrange("p (b j) -> p b j", b=bnw),
                stwv.rearrange("p (b j) -> p b j", b=bnw),
                cols["gate"][0:P, cslice].unsqueeze(2)
                .broadcast_to([P, bnw, kkc]),
                ALU.mult)
            pp = psC.tile([128, 512], F32, tag="Cp")
            for gg in range(gn):
                g = g0 + gg
                slp = pp[:, gg * kkc:(gg + 1) * kkc]
                for cs in range(cpg):
                    nc.tensor.matmul(
                        slp, oblk(g, cs),
                        stw[0:P, (gg * cpg + cs) * kkc:
                            (gg * cpg + cs + 1) * kkc],
                        start=(cs == 0), stop=(cs == cpg - 1))
            accum_readout(pp, gn, g0, kkc, "com")

        # ---------- next-stage state ----------
        if not last:
            Pn = STAGES[s_i + 1]["P"]
            bigXn = big1.tile([128, G * 128], F32, tag="Xn")
            for g in range(G):
                ppx = psC.tile([128, 512], F32, tag="Cp")
                for cs in range(cpg):
                    nc.tensor.matmul(
                        ppx[0:k, 0:128], stgblk(g, cs), oblk(g, cs),
                        start=(cs == 0), stop=(cs == cpg - 1))
                nc.scalar.copy(bigXn[0:Pn, g * 128:(g + 1) * 128],
                               ppx[0:k, 0:128])

            # pooled adjacency: A2 = S A S^T per graph, then transpose
            if s_i == 0:
                bigATn = big1.tile([128, G * 128], F32, tag="ATn2")
            else:
                bigATn = big1.tile([64, G * 64], F32, tag="ATn3")
            dgi = work.tile([128, 32], F32, tag="dgi")
            stu = None
            for g in range(G):
                if g % 4 == 0:
                    if s_i == 0:
                        atw2 = at_load(g, 4)
                    # ungated selection for the adjacency chain
                    stu = wave.tile([128, 8 * 128], F32, tag="zw")
                    bnw = 4 * cpg
                    nc.vector.tensor_tensor(
                        stu[0:P, 0:bnw * k].rearrange("p (b j) -> p b j",
                                                      b=bnw),
                        iota[0:P, 0:k].unsqueeze(1).broadcast_to([P, bnw, k]),
                        cols["dest"][0:P, g * cpg:g * cpg + bnw].unsqueeze(2)
                        .broadcast_to([P, bnw, k]),
                        ALU.is_equal)

                def stublk(gg, cs, _s=stu, _g0=(g // 4) * 4, _cpg=cpg,
                           _P=P, _k=k):
                    o = ((gg - _g0) * _cpg + cs) * _k
                    return _s[0:_P, o:o + _k]

                ta_src = atw2 if s_i == 0 else bigAT
                tag0 = (g // 4) * 4 if s_i == 0 else 0
                ta_sb = smal.tile([128, 128], F32, tag="ta_sb")
                pa2 = psS.tile([128, 512], F32, tag="S")
                for cd in range(cpg):
                    pta = psT.tile([128, 128], F32, tag="T")
                    for cs in range(cpg):
                        nc.tensor.matmul(
                            pta[0:P, 0:k],
                            atblk_of(ta_src, tag0, g, cs, cd), stublk(g, cs),
                            start=(cs == 0), stop=(cs == cpg - 1))
                    nc.scalar.copy(ta_sb[0:P, 0:k], pta[0:P, 0:k])
                    nc.tensor.matmul(pa2[0:k, 0:k], stublk(g, cd),
                                     ta_sb[0:P, 0:k],
                                     start=(cd == 0), stop=(cd == cpg - 1))
                a2 = smal.tile([128, 128], F32, tag="a2t")
                nc.scalar.copy(a2[0:k, 0:k], pa2[0:k, 0:k])
                nc.vector.tensor_reduce(dgi[0:k, g:g + 1], a2[0:k, 0:k],
                                        AX.X, ALU.add)
                pat = psT.tile([128, 128], F32, tag="T")
                nc.tensor.transpose(pat[0:k, 0:k], a2[0:k, 0:k],
                                    id128[0:k, 0:k])
                nc.scalar.copy(bigATn[0:k, g * k:(g + 1) * k], pat[0:k, 0:k])

            dgo = work.tile([128, 32], F32, tag="dgo")
            nc.vector.tensor_reduce(
                dgo[0:k, :],
                bigATn[0:k, 0:G * k].rearrange("p (g d) -> p g d", g=G),
                AX.X, ALU.add)
            ron = rsqrt_of(dgo[0:k, :], [k, G], f"r_o{s_i + 2}")
            rin = rsqrt_of(dgi[0:k, :], [k, G], f"r_i{s_i + 2}")
            prb = psT.tile([128, 128], F32, tag="T")
            nc.tensor.transpose(prb[0:32, 0:k], rin[:], id128[0:k, 0:k])
            rinb = work.tile([32, 128], F32, tag="rinb")
            nc.scalar.copy(rinb[:, 0:k], prb[0:32, 0:k])

            if s_i == 0:
                dbg("x2", bigXn[0:128, 0:128])
                dbg("at2", bigATn[0:128, 0:128])
            else:
                dbg("x3", bigXn[0:64, 0:128])
                dbg("at3", bigATn[0:64, 0:64])
            st = {"X": bigXn, "AT": bigATn, "ro_c": ron, "ri_c": rin,
                  "ri_b": rinb[:, 0:k]}
        else:
            bigO_last = bigO

    # ---------------- graph MLPs (feature-major) ----------------
    def graph_mlp(avg_t, max_t, out_d, tagp):
        ph = psS.tile([128, 512], F32, tag="S")
        nc.tensor.matmul(ph[:, 0:32], W("Wg0a"), avg_t[:], start=True,
                         stop=False)
        nc.tensor.matmul(ph[:, 0:32], W("Wg0b"), max_t[:], start=False,
                         stop=True)
        h1 = work.tile([128, 32], F32, tag=f"{tagp}h1")
        nc.scalar.activation(h1[:], ph[:, 0:32], AF.Relu,
                             bias=bias_cols["bg0"][:])
        ph2 = psS.tile([128, 512], F32, tag="S")
        nc.tensor.matmul(ph2[0:64, 0:32], W("Wg1"), h1[:], start=True,
                         stop=True)
        h2 = work.tile([64, 32], F32, tag=f"{tagp}h2")
        nc.scalar.activation(h2[:], ph2[0:64, 0:32], AF.Relu,
                             bias=bias_cols["bg1"][:])
        ph3 = psS.tile([128, 512], F32, tag="S")
        nc.tensor.matmul(ph3[0:10, 0:32], W("Wg2"), h2[:], start=True,
                         stop=True)
        o = work.tile([10, 32], F32, tag=f"{tagp}o")
        nc.scalar.activation(o[:], ph3[0:10, 0:32], AF.Identity,
                             bias=bias_cols["bg2"][:])
        nc.sync.dma_start(out_d[:], o[:])

    dbg("hgdavg", hgT[("dis", "avg")][:])
    dbg("hgdmax", hgT[("dis", "max")][:])
    dbg("hgcavg", hgT[("com", "avg")][:])
    dbg("hgcmax", hgT[("com", "max")][:])
    graph_mlp(hgT[("dis", "avg")], hgT[("dis", "max")], scoresT_d, "gmd")
    graph_mlp(hgT[("com", "avg")], hgT[("com", "max")], scorescT_d, "gmc")

    nc.sync.dma_start(hg3comT_d[0:128, :], hg3c_avg[:])
    nc.sync.dma_start(hg3comT_d[128:256, :], hg3c_max[:])

    # ---------------- node MLP on out3 (graph pairs, feature-major) -------
    npredT = work.tile([10, G * 64], F32, tag="npredT")
    for gp in range(G // 2):
        pot = psS.tile([128, 512], F32, tag="S")
        for hh in range(2):
            g = gp * 2 + hh
            nc.tensor.transpose(pot[:, hh * 64:(hh + 1) * 64],
                                bigO_last[0:64, g * 128:(g + 1) * 128],
                                id128[0:64, 0:64])
        o3T = smal.tile([128, 128], F32, tag="o3T")
        nc.scalar.copy(o3T[:], pot[:, 0:128])
        p1 = psS.tile([128, 512], F32, tag="S")
        nc.tensor.matmul(p1[0:64, 0:128], W("Wp0"), o3T[:], start=True,
                         stop=True)
        h1 = smal.tile([64, 128], F32, tag="np_h1")
        nc.scalar.activation(h1[:], p1[0:64, 0:128], AF.Relu,
                             bias=bias_cols["bp0"][:])
        p2 = psS.tile([128, 512], F32, tag="S")
        nc.tensor.matmul(p2[0:32, 0:128], W("Wp1"), h1[:], start=True,
                         stop=True)
        h2 = smal.tile([32, 128], F32, tag="np_h2")
        nc.scalar.activation(h2[:], p2[0:32, 0:128], AF.Relu,
                             bias=bias_cols["bp1"][:])
        p3 = psS.tile([128, 512], F32, tag="S")
        nc.tensor.matmul(p3[0:10, 0:128], W("Wp2"), h2[:], start=True,
                         stop=True)
        nc.scalar.activation(npredT[:, gp * 128:(gp + 1) * 128],
                             p3[0:10, 0:128], AF.Identity,
                             bias=bias_cols["bp2"][:])
    nc.sync.dma_start(npredT_d[:], npredT[:])

    nc.sync.dma_start(e1_d[:], e1_tile[:])
    nc.sync.dma_start(msum_d[:], msum_tile[:])


# --------------------------------------------------------------------------
# host side
# --------------------------------------------------------------------------

_CACHE = {}


def _build_program():
    nc = bacc.Bacc("TRN2", target_bir_lowering=False, debug=False,
                   num_devices=NCORES)
    ins = [
        nc.dram_tensor("bigx", [128, 64 * 128], F32,
                       kind="ExternalInput").ap(),
        nc.dram_tensor("bigat", [128, 64 * 256], F32,
                       kind="ExternalInput").ap(),
        nc.dram_tensor("dego_col", [128, 64], F32, kind="ExternalInput").ap(),
        nc.dram_tensor("degi_col", [128, 64], F32, kind="ExternalInput").ap(),
        nc.dram_tensor("degi_byg", [32, 256], F32, kind="ExternalInput").ap(),
        nc.dram_tensor("wpack", [128, WCOLS], F32, kind="ExternalInput").ap(),
        nc.dram_tensor("bpack", [1, BCOLS], F32, kind="ExternalInput").ap(),
        nc.dram_tensor("gsel", [32, 128], F32, kind="ExternalInput").ap(),
        nc.dram_tensor("id128", [128, 128], F32, kind="ExternalInput").ap(),
        nc.dram_tensor("iota", [128, 128], F32, kind="ExternalInput").ap(),
    ]
    outs = [
        nc.dram_tensor("scoresT", [10, 32], F32, kind="ExternalOutput").ap(),
        nc.dram_tensor("scorescT", [10, 32], F32, kind="ExternalOutput").ap(),
        nc.dram_tensor("hg3comT", [256, 32], F32, kind="ExternalOutput").ap(),
        nc.dram_tensor("npredT", [10, G * 64], F32,
                       kind="ExternalOutput").ap(),
        nc.dram_tensor("e1", [32, 256], F32, kind="ExternalOutput").ap(),
        nc.dram_tensor("msum", [1, 2], F32, kind="ExternalOutput").ap(),
    ]
    dbg_aps = {}
    for nm in DEBUG:
        dbg_aps[nm] = nc.dram_tensor(f"dbg_{nm}", DEBUG_SHAPES[nm], F32,
                                     kind="ExternalOutput").ap()
    with tile.TileContext(nc) as tc:
        with ExitStack() as ctx:
            build_body(ctx, tc, outs, ins, dbg_aps)
    nc.compile()
    return nc


def host_prep(inputs):
    """Per-core input maps (sharding + index-only preprocessing)."""
    feature = np.asarray(inputs["feature"], np.float32)
    src = np.asarray(inputs["src"], np.int64)
    dst = np.asarray(inputs["dst"], np.int64)

    wp = np.zeros((128, WCOLS), np.float32)
    for nm, rows, cols in WSIZES:
        if nm == "Wg0a":
            v = np.asarray(inputs["Wg0"][0:128], np.float32)
        elif nm == "Wg0b":
            v = np.asarray(inputs["Wg0"][128:256], np.float32)
        else:
            v = np.asarray(inputs[nm], np.float32).reshape(rows, cols)
        wp[0:rows, WOFF[nm]:WOFF[nm] + cols] = v
    bp = np.zeros((1, BCOLS), np.float32)
    for nm, colsn in BSIZES:
        bp[0, BOFF[nm]:BOFF[nm] + colsn] = np.asarray(
            inputs[nm], np.float32).reshape(-1)

    gsel = np.zeros((32, 128), np.float32)
    for g in range(32):
        gsel[g, g * 4:(g + 1) * 4] = 1.0
    id128 = np.eye(128, dtype=np.float32)
    iota = np.tile(np.arange(128, dtype=np.float32), (128, 1))

    in_maps = []
    for c in range(NCORES):
        Xc = feature[c * G * NP:(c + 1) * G * NP].reshape(G, 2, 128, D)
        bigx = np.ascontiguousarray(
            Xc.transpose(2, 0, 1, 3).reshape(128, 64 * 128))
        e0, e1e = c * G * EPG, (c + 1) * G * EPG
        s_loc = src[e0:e1e] - c * G * NP
        d_loc = dst[e0:e1e] - c * G * NP
        g_loc = s_loc // NP
        key = (g_loc * NP + (s_loc % NP)) * NP + (d_loc % NP)
        AT = np.bincount(key, minlength=G * NP * NP).reshape(G, NP, NP)
        AT = AT.astype(np.float32)
        bigat = np.ascontiguousarray(
            AT.reshape(G, 2, 128, 2, 128).transpose(2, 0, 1, 3, 4)
            .reshape(128, 64 * 256))
        deg_out = AT.sum(axis=2)
        deg_in = AT.sum(axis=1)
        dego_col = np.ascontiguousarray(
            deg_out.reshape(G, 2, 128).transpose(2, 0, 1).reshape(128, 64))
        degi_col = np.ascontiguousarray(
            deg_in.reshape(G, 2, 128).transpose(2, 0, 1).reshape(128, 64))
        in_maps.append({
            "bigx": bigx, "bigat": bigat,
            "dego_col": dego_col, "degi_col": degi_col,
            "degi_byg": np.ascontiguousarray(deg_in),
            "wpack": wp, "bpack": bp, "gsel": gsel, "id128": id128,
            "iota": iota,
        })
    return in_maps


def unshard(results):
    rs = results
    scores = np.concatenate([rs[c]["scoresT"].T for c in range(NCORES)], 0)
    scores_com = np.concatenate(
        [rs[c]["scorescT"].T for c in range(NCORES)], 0)
    hg3_com = np.concatenate([rs[c]["hg3comT"].T for c in range(NCORES)], 0)
    node_pred = np.concatenate([rs[c]["npredT"].T for c in range(NCORES)], 0)
    Ms = np.array([rs[c]["msum"][0, 0] for c in range(NCORES)])
    Ss = np.array([rs[c]["msum"][0, 1] for c in range(NCORES)])
    Mg = Ms.max()
    Sg = float((Ss * np.exp(Ms - Mg)).sum())
    node_score1 = np.concatenate(
        [rs[c]["e1"].reshape(-1) * (np.exp(Ms[c] - Mg) / Sg)
         for c in range(NCORES)])
    return (scores.astype(np.float32), scores_com.astype(np.float32),
            hg3_com.astype(np.float32), node_pred.astype(np.float32),
            node_score1.astype(np.float32))


def kernel_raw(**inputs):
    if "nc" not in _CACHE:
        _CACHE["nc"] = _build_program()
    nc = _CACHE["nc"]
    in_maps = host_prep(inputs)
    res = bass_utils.run_bass_kernel_spmd(nc, in_maps,
                                          core_ids=list(range(NCORES)))
    return res.results


def kernel(**inputs):
    if "nc" not in _CACHE:
        _CACHE["nc"] = _build_program()
    nc = _CACHE["nc"]
    in_maps = host_prep(inputs)
    res = bass_utils.run_bass_kernel_spmd(nc, in_maps,
                                          core_ids=list(range(NCORES)))
    return unshard(res.results)
